# revision 1
# baseline (speedup 1.0000x reference)
"""CapsuleLayer dynamic-routing kernel for 8 Trainium2 NeuronCores.

Math (per sample):
    u_hat[n,m,c] = sum_d x[n,d] W[d,m,c]           (never materialized)
    routing r=1..3:
        c = softmax_n(b)            -> c_unnorm = exp(b), Z[m] = sum_n c_unnorm
        s[m,c] = sum_n c[n,m] u_hat[n,m,c] = (sum_d T[m,d] W[d,m,c]) / Z[m]
                 where T[m,d] = sum_n c_unnorm[n,m] x[n,d]
        v = squash(s)
        b += sum_c v[m,c] u_hat[n,m,c] = x @ P_r.T  where P_r[m,d] = sum_c v W
    With Q_r = sum_{r'<=r} P_r', the logits are always b_r = x @ Q_r.T, so we
    accumulate Q (tiny) instead of b (big).  exp() overflow-safe without the
    max-subtraction: |b| <~ 60 << 88.

Sharding: batch 64 -> 8 samples/core, fully independent.
"""

import os
import sys

import numpy as np

for _p in ("/opt/trn_rl_repo", os.path.expanduser("~/.axon_site/_ro/trn_rl_repo")):
    if os.path.isdir(_p) and _p not in sys.path:
        sys.path.insert(0, _p)

import concourse.bass as bass
import concourse.tile as tile
from concourse import mybir
from concourse.vector_clock import ScopedClock, VectorClock
from bass_rust import N_PROCS


class _SplitDrainTC(tile.TileContext):
    """TileContext whose exit drain is split into several drains with few
    sem waits each: walrus rejects a single drain waiting on >~8 sems."""

    def _drain_and_barrier(self, tick_clock, wait_clock):
        gc = tick_clock.global_clock
        CH = 1
        for i in range(0, N_PROCS, CH):
            sub = VectorClock(
                [gc[p] if i <= p < i + CH else 0 for p in range(N_PROCS)]
            )
            drain_inst = self.nc.sync.drain()
            wait_clock.add_sem_waits(
                drain_inst.ins, ScopedClock({None: sub})
            )
        self.nc.all_engine_barrier()
        assert self.sems is not None
        popped = self.nc._tile_sem_poison_stack.pop()
        assert popped is self._sem_poison
        self.nc.clear_and_free_semaphores(list(self.sems.allocated().values()))
        self.nc.all_engine_barrier()

B, N, D, M, C = 64, 2048, 16, 32, 16
NCORES = 8
BL = B // NCORES          # samples per core = 8
G = BL // 4               # sample groups of 4 -> 2
NCHUNK = N // 128         # 16
NWIN = N // 256           # 8 transpose windows of 256
ROUTINGS = 3
EPS = 1e-7
F32 = mybir.dt.float32
BF16 = mybir.dt.bfloat16
ALU = mybir.AluOpType
ACTF = mybir.ActivationFunctionType
RSQRT_MAGIC = 0x5F3759DF


def _bcast(ap, idx, num):
    """Insert a stride-0 free dim of size `num` at free-dim position idx."""
    dims = list(ap.ap)
    dims.insert(1 + idx, [0, num])
    return bass.AP(ap.tensor, ap.offset, dims)


def build_bass():
    nc = bass.Bass()
    x_in = nc.declare_dram_parameter("x", [BL, N, D], F32, isOutput=False)
    w_in = nc.declare_dram_parameter("w", [D, M, C], F32, isOutput=False)
    v_out = nc.declare_dram_parameter("v", [BL, M, C], F32, isOutput=True)

    with _SplitDrainTC(nc) as tc:
        _emit(tc, x_in, w_in, v_out)
    return nc


def _emit(tc, x_in, w_in, v_out):
    nc = tc.nc
    P = 128

    from contextlib import ExitStack

    ctx = ExitStack()
    const = ctx.enter_context(tc.tile_pool(name="const", bufs=1))
    ld = ctx.enter_context(tc.tile_pool(name="ld", bufs=2))
    work = ctx.enter_context(tc.tile_pool(name="work", bufs=2))
    small = ctx.enter_context(tc.tile_pool(name="small", bufs=4))
    psum_b = ctx.enter_context(tc.tile_pool(name="psum_b", bufs=2, space="PSUM"))
    psum_t = ctx.enter_context(tc.tile_pool(name="psum_t", bufs=3, space="PSUM"))

    x = x_in[:]
    w = w_in[:]
    vout = v_out[:]

    # ---------------- constants / input staging ----------------
    # xq[g][p, k, si, 0:16] = x[4g+si, 128k+p, d]; [..,16] = 1.0 (Z column).
    # Staged via a raw tile + one flat 2D copy so every consumer instruction
    # waits on a single DVE semaphore and lowers to a 2D (wait-slot-rich)
    # encoding: walrus rejects >=3D instructions with multiple sync waits.
    xq = []
    for g in range(G):
        xq_raw = const.tile(
            [P, NCHUNK, 4, D], F32, name=f"xq_raw_{g}", tag=f"xq_raw_{g}"
        )
        eng = nc.scalar if g == 0 else nc.sync
        for si in range(4):
            eng.dma_start(
                out=xq_raw[:, :, si, :],
                in_=x[4 * g + si].rearrange("(k p) d -> p k d", p=P),
            )
        xqg = const.tile([P, NCHUNK, 4, D + 1], F32, name=f"xq_{g}", tag=f"xq_{g}")
        nc.gpsimd.memset(xqg[:, :, :, D : D + 1], 1.0)
        for si in range(4):
            nc.gpsimd.tensor_copy(
                out=xqg[:, :, si, 0:D], in_=xq_raw[:, :, si, :]
            )
        xq.append(xqg)

    # W4[32g+m, d, c] = W[d, m, c] replicated over the 4 samples of a group
    w1 = const.tile([32, D, C], F32)
    nc.scalar.dma_start(out=w1, in_=w.rearrange("d m c -> m d c"))
    w4 = const.tile([P, D, C], F32)
    for gi in range(4):
        nc.vector.tensor_copy(out=w4[32 * gi : 32 * gi + 32, :, :], in_=w1)

    cconst = const.tile([P, P], F32)
    nc.gpsimd.memset(cconst, 1.0 / N)
    zz = const.tile([P, 2], BF16)
    nc.gpsimd.memset(zz, 0.0)
    zb = const.tile([P, P], BF16)
    nc.gpsimd.memset(zb, 0.0)
    wtz = psum_t.tile([P, 72], F32, name="wtz", tag="tz")
    nc.tensor.matmul(out=wtz[:, 68:70], lhsT=zb[:, :], rhs=zz[:, :],
                     start=True, stop=True)

    # xT4[g][32s'+d, n] = x[4g+s', n, d]  (rows with d in [16,32) are zero)
    # built via DVE 32x32 block transpose of [32s'+nl, (win, wv, 32dpad)] tiles
    xt4 = [const.tile([P, N], F32, name=f"xt4_{g}", tag=f"xt4_{g}") for g in range(G)]
    for g in range(G):
        lt = ld.tile([P, NWIN, 8, 32], F32, name="ldt", tag="ldt")
        for si in range(4):
            eng = nc.sync if si < 2 else nc.scalar
            s = 4 * g + si
            eng.dma_start(
                out=lt[32 * si : 32 * si + 32, :, :, 0:D].rearrange(
                    "nl win wv d -> nl (win wv) d"
                ),
                in_=x[s].rearrange("(wvs nl) d -> nl wvs d", nl=32),
            )
        lt2 = ld.tile([P, NWIN, 8, 32], F32, name="ldt2", tag="ldt2")
        ceng = nc.scalar if g == 0 else nc.gpsimd
        for si in range(4):
            # staging copies: each waits on exactly one DMA queue semaphore
            cop = ceng.copy if g == 0 else ceng.tensor_copy
            cop(
                out=lt2[32 * si : 32 * si + 32, :, :, 0:D].rearrange(
                    "nl win wv d -> nl (win wv) d"
                ),
                in_=lt[32 * si : 32 * si + 32, :, :, 0:D].rearrange(
                    "nl win wv d -> nl (win wv) d"
                ),
            )
        nc.vector.tensor_copy(out=lt2[:, :, :, D:32], in_=lt2[:, :, :, 0:D])
        for win in range(NWIN):
            # block transpose: out[32si+d, 32wv+nl] = lt2[32si+nl, win, wv, d]
            nc.vector.transpose(
                out=xt4[g][:, 256 * win : 256 * (win + 1)],
                in_=lt2[:, win, :, :].rearrange("p wv d -> p (wv d)"),
            )

    # bf16 hi/lo packed xT4: rows 32si+d hold -x_hi, rows 32si+16+d hold
    # x_lo.  With QA rows (-Qhi | +Qhi) and QB rows (-Qlo | 0):
    # b = xHL@QA + xHL@QB = xhi@Qhi + xlo@Qhi + xhi@Qlo   (~1e-5 rel)
    maskLO = const.tile([P, 1], F32)
    sgn = const.tile([P, 1], F32)
    posz = const.tile([P, 1], F32)
    nc.vector.memset(maskLO, 1.0)
    nc.vector.memset(sgn, 1.0)
    nc.vector.memset(posz, 0.0)
    for si in range(4):
        nc.vector.memset(maskLO[32 * si : 32 * si + D, :], 0.0)
        nc.vector.memset(sgn[32 * si : 32 * si + D, :], -1.0)
        nc.vector.memset(posz[32 * si : 32 * si + D, :], 1.0)
    xt4hl = [
        const.tile([P, N], BF16, name=f"xt4hl_{g}", tag=f"xt4hl_{g}")
        for g in range(G)
    ]
    for g in range(G):
        xthb = small.tile([P, N], BF16, tag="xthb")
        H = N // 2
        for hh in range(2):
            sl = slice(hh * H, (hh + 1) * H)
            nc.vector.tensor_copy(out=xthb[:, sl], in_=xt4[g][:, sl])
            nc.vector.scalar_tensor_tensor(
                out=xt4hl[g][:, sl], in0=xt4[g][:, sl], scalar=maskLO,
                in1=xthb[:, sl], op0=ALU.mult, op1=ALU.subtract,
            )

    # Q_bd[g]: [128, 128]; rows 32s'+d (d<16) hold Q_s^T, cols 32s'+m
    qbd = [const.tile([P, P], F32, name=f"qbd_{g}", tag=f"qbd_{g}") for g in range(G)]
    qa = [const.tile([P, P], BF16, name=f"qa_{g}", tag=f"qa_{g}") for g in range(G)]
    qb = [const.tile([P, P], BF16, name=f"qb_{g}", tag=f"qb_{g}") for g in range(G)]
    for g in range(G):
        nc.vector.memset(qbd[g], 0.0)

    # ---------------- routing iterations ----------------
    for r in range(1, ROUTINGS + 1):
        c_sb = {}
        for g in range(G):
            if r > 1:
                # logits b = x @ Q_{r-1}, then c_unnorm = exp(b)
                # c_sb[g][p, k, 32s'+m] = exp(b)[128k+p, (s',m)]
                c_sb[g] = work.tile([P, NCHUNK, P], F32, name=f"c_{g}", tag=f"c_{g}")
                for half in range(NCHUNK // 8):  # 2 psum tiles of 8 chunks
                    bp = psum_b.tile([P, 8, P], F32, tag="bpsum")
                    # absorber: zero-writing opener takes the PE drain wait
                    nc.tensor.matmul(out=bp[:, 0, :], lhsT=zb[:, :],
                                     rhs=zb[:, :], start=True, stop=False)
                    for i in range(8):
                        k = 8 * half + i
                        for j, rh in enumerate((qa[g], qb[g])):
                            nc.tensor.matmul(
                                out=bp[:, i, :],
                                lhsT=xt4hl[g][:, 128 * k : 128 * (k + 1)],
                                rhs=rh[:, :],
                                start=(j == 0 and i > 0),
                                stop=(j == 1),
                            )
                    nc.scalar.activation(
                        out=c_sb[g][:, 8 * half : 8 * half + 8, :].rearrange(
                            "p k f -> p (k f)"
                        ),
                        in_=bp[:, :, :].rearrange("p k f -> p (k f)"),
                        func=ACTF.Exp,
                    )

            # ---- T[m,d] and Z accumulated over n-chunks on PE ----
            # one accumulation group per bank: lhsT = 4 samples' c blocks,
            # rhs = the same 4 samples' [x|1] quads; diagonal blocks extracted
            tz = psum_t.tile([P, 72], F32, tag="tz")
            # absorber A: takes the PE psum-slot drain wait (opens group).
            # lhsT=qlo delays its readiness past the logits matmuls so the
            # scheduler doesn't hoist it before the DVE tick is absorbed.
            a_lhs = qb[g] if r > 1 else zb
            nc.tensor.matmul(out=tz[:, 68:70], lhsT=a_lhs[:, :],
                             rhs=zz[:, :], start=True, stop=False)
            if r > 1:
                # absorber B: takes the ACT (exp) wait
                cb = c_sb[g][:, 0, 0:2]
                nc.tensor.matmul(out=tz[0:2, 70:72], lhsT=cb, rhs=cb,
                                 start=False, stop=False)
            for k in range(NCHUNK):
                lhsT = cconst[:, :] if r == 1 else c_sb[g][:, k, :]
                nc.tensor.matmul(
                    out=tz[:, 0:68],
                    lhsT=lhsT,
                    rhs=xq[g][:, k, :, :].rearrange("p s f -> p (s f)"),
                    start=False,
                    stop=(k == NCHUNK - 1),
                )

            t4 = small.tile([P, D + 1], F32, tag="t4")
            for si in range(4):
                nc.vector.tensor_copy(
                    out=t4[32 * si : 32 * si + 32, :],
                    in_=tz[32 * si : 32 * si + 32, 17 * si : 17 * si + 17],
                )
            rz = small.tile([P, 1], F32, tag="rz")
            nc.vector.reciprocal(out=rz, in_=t4[:, D : D + 1])

            # ---- s[m,c] = (sum_d T[m,d] W[d,m,c]) / Z ----
            prod = small.tile([P, D, C], F32, tag="prod")
            nc.vector.tensor_tensor(
                out=prod[:, :, :],
                in0=_bcast(t4[:, 0:D], 1, C),
                in1=w4[:, :, :],
                op=ALU.mult,
            )
            s4 = small.tile([P, C], F32, tag="s4")
            nc.vector.tensor_reduce(
                out=s4[:, :],
                in_=prod[:, :, :].rearrange("p d c -> p c d"),
                axis=mybir.AxisListType.X,
                op=ALU.add,
            )
            nc.vector.tensor_scalar_mul(out=s4[:, :], in0=s4[:, :], scalar1=rz)

            # ---- squash ----
            n2 = small.tile([P, 1], F32, tag="n2")
            sq = small.tile([P, C], F32, tag="sq")
            nc.vector.scalar_tensor_tensor(
                out=sq[:, :],
                in0=s4[:, :],
                scalar=1.0,
                in1=s4[:, :],
                op0=ALU.mult,
                op1=ALU.mult,
                accum_out=n2,
            )
            # y ~= rsqrt(n2) : magic seed + 3 Newton steps (no ACT table switch)
            y = small.tile([P, 1], F32, tag="y")
            hlf = small.tile([P, 1], F32, tag="hlf")
            nc.vector.tensor_scalar(
                out=y.bitcast(mybir.dt.int32),
                in0=n2.bitcast(mybir.dt.int32),
                scalar1=1,
                scalar2=None,
                op0=ALU.logical_shift_right,
            )
            nc.vector.tensor_scalar(
                out=y.bitcast(mybir.dt.int32),
                in0=y.bitcast(mybir.dt.int32),
                scalar1=-1,
                scalar2=RSQRT_MAGIC,
                op0=ALU.mult,
                op1=ALU.add,
            )
            for _ in range(2):
                nc.vector.tensor_mul(out=hlf, in0=y, in1=y)
                nc.vector.tensor_mul(out=hlf, in0=hlf, in1=n2)
                nc.vector.tensor_scalar(
                    out=hlf,
                    in0=hlf,
                    scalar1=-0.5,
                    scalar2=1.5,
                    op0=ALU.mult,
                    op1=ALU.add,
                )
                nc.vector.tensor_mul(out=y, in0=y, in1=hlf)
            nrm = small.tile([P, 1], F32, tag="nrm")
            nc.vector.tensor_mul(out=nrm, in0=n2, in1=y)  # = sqrt(n2)
            one_p_n2 = small.tile([P, 1], F32, tag="opn")
            nc.vector.tensor_scalar_add(out=one_p_n2, in0=n2, scalar1=1.0)
            nc.vector.tensor_scalar_add(out=nrm, in0=nrm, scalar1=EPS)
            den = small.tile([P, 1], F32, tag="den")
            nc.vector.tensor_mul(out=den, in0=one_p_n2, in1=nrm)
            nc.vector.reciprocal(out=den, in_=den)
            fct = small.tile([P, 1], F32, tag="fct")
            nc.vector.tensor_mul(out=fct, in0=n2, in1=den)
            v4 = small.tile([P, C], F32, tag="v4")
            nc.vector.tensor_scalar_mul(out=v4[:, :], in0=s4[:, :], scalar1=fct)

            if r == ROUTINGS:
                vstage = small.tile([P, C], F32, tag="vstage")
                nc.gpsimd.tensor_copy(out=vstage[:, :], in_=v4[:, :])
                nc.gpsimd.dma_start(
                    out=vout[4 * g : 4 * g + 4].rearrange("s m c -> (s m) c"),
                    in_=vstage[:, :],
                )
                continue

            # ---- P[m,d] = sum_c v[m,c] W[d,m,c]; Q += P^T (block diag) ----
            prod2 = small.tile([P, D, C], F32, tag="prod2")
            nc.vector.tensor_tensor(
                out=prod2[:, :, :],
                in0=_bcast(v4[:, :], 0, D),
                in1=w4[:, :, :],
                op=ALU.mult,
            )
            p4 = small.tile([P, 32], F32, tag="p4")
            nc.vector.tensor_reduce(
                out=p4[:, 0:D],
                in_=prod2[:, :, :],
                axis=mybir.AxisListType.X,
                op=ALU.add,
            )
            nc.vector.tensor_copy(out=p4[:, D:32], in_=p4[:, 0:D])
            # DVE 32x32 block transpose: p4t[32si+d, m] = p4[32si+m, d]
            p4t = small.tile([P, 32], F32, tag="p4t")
            nc.vector.transpose(out=p4t[:, :], in_=p4[:, :])
            for si in range(4):
                srcb = p4t[32 * si : 32 * si + 32, 0:32]
                dst = qbd[g][32 * si : 32 * si + 32, 32 * si : 32 * si + 32]
                if r == 1:
                    nc.vector.tensor_copy(out=dst, in_=srcb)
                else:
                    nc.vector.tensor_add(out=dst, in0=dst, in1=srcb)
            # QA = bf16(qbd)*sgn; QB = bf16(qh - qbd) masked to hi rows
            nc.vector.tensor_scalar_mul(out=qa[g][:, :], in0=qbd[g][:, :],
                                        scalar1=sgn)
            mlq = small.tile([P, P], F32, tag="mlq")
            nc.vector.scalar_tensor_tensor(
                out=mlq[:, :], in0=qa[g][:, :], scalar=sgn,
                in1=qbd[g][:, :], op0=ALU.mult, op1=ALU.subtract,
            )
            nc.vector.tensor_scalar_mul(out=qb[g][:, :], in0=mlq[:, :],
                                        scalar1=posz)

    ctx.close()


_NC_CACHE = None
_RUNNER = None


def _get_nc():
    global _NC_CACHE
    if _NC_CACHE is None:
        _NC_CACHE = build_bass()
    return _NC_CACHE


def _get_runner():
    """Build the sharded jitted executable once and reuse it across calls
    (run_bass_kernel_spmd re-traces jax on every invocation)."""
    global _RUNNER
    if _RUNNER is not None:
        return _RUNNER
    import jax
    import jax.numpy as jnp
    from jax.sharding import Mesh, PartitionSpec
    from jax.experimental.shard_map import shard_map
    from concourse import bass2jax, mybir as mb
    from concourse.bass2jax import (
        _bass_exec_p,
        install_neuronx_cc_hook,
        partition_id_tensor,
    )

    install_neuronx_cc_hook()
    nc = _get_nc()

    part_name = nc.partition_id_tensor.name if nc.partition_id_tensor else None
    in_names, out_names, out_avals, zero_outs = [], [], [], []
    for alloc in nc.m.functions[0].allocations:
        if not isinstance(alloc, mb.MemoryLocationSet):
            continue
        name = alloc.memorylocations[0].name
        if alloc.kind == "ExternalInput":
            if name != part_name:
                in_names.append(name)
        elif alloc.kind == "ExternalOutput":
            out_names.append(name)
            shape = tuple(alloc.tensor_shape)
            dtype = mb.dt.np(alloc.dtype)
            out_avals.append(jax.core.ShapedArray(shape, dtype))
            zero_outs.append(np.zeros(shape, dtype))
    n_params = len(in_names)
    all_names = in_names + out_names
    if part_name is not None:
        all_names.append(part_name)

    def _body(*args):
        operands = list(args)
        if part_name is not None:
            operands.append(partition_id_tensor())
        outs = _bass_exec_p.bind(
            *operands,
            out_avals=tuple(out_avals),
            in_names=tuple(all_names),
            out_names=tuple(out_names),
            lowering_input_output_aliases=(),
            sim_require_finite=True,
            sim_require_nnan=True,
            nc=nc,
        )
        return tuple(outs)

    devices = jax.devices()[:NCORES]
    mesh = Mesh(np.asarray(devices), ("core",))
    n_outs = len(out_names)
    sharded = jax.jit(
        shard_map(
            _body,
            mesh=mesh,
            in_specs=(PartitionSpec("core"),) * (n_params + n_outs),
            out_specs=(PartitionSpec("core"),) * n_outs,
            check_rep=False,
        ),
        donate_argnums=tuple(range(n_params, n_params + n_outs)),
        keep_unused=True,
    )
    _RUNNER = (sharded, in_names, zero_outs)
    return _RUNNER


def kernel(inputs: np.ndarray, W: np.ndarray) -> np.ndarray:
    inputs = np.ascontiguousarray(np.asarray(inputs, dtype=np.float32))
    W = np.ascontiguousarray(np.asarray(W, dtype=np.float32))
    sharded, in_names, zero_outs = _get_runner()
    per_name = {
        "x": inputs.reshape(NCORES * BL, N, D),
        "w": np.concatenate([W] * NCORES, axis=0),
    }
    concat_in = [per_name[n] for n in in_names]
    concat_zeros = [
        np.zeros((NCORES * z.shape[0], *z.shape[1:]), z.dtype) for z in zero_outs
    ]
    out_arrs = sharded(*concat_in, *concat_zeros)
    return np.asarray(out_arrs[0]).reshape(B, M, C).astype(np.float32)

